# revision 11
# baseline (speedup 1.0000x reference)
"""Complex BatchNorm2d on 8 TRN2 NeuronCores (Bass/Tile).

Reference op: per-channel complex whitening + affine over [B=8, C=64, H=256,
W=256, 2] fp32 (last dim = real/imag interleaved).

Sharding: channel-parallel. C=64 -> 8 channels per core; each core sees ALL
batches for its channels, so per-channel statistics are fully local and no
collective is needed (the batch axis is reduced on-core).

Per core:
  pass 1 (stats): stream the 33.5 MB local shard through SBUF.
     bn_stats computes mean/var of even-index (real) and odd-index (imag)
     elements separately per 512-float window -> exactly the interleaved
     layout.  tensor_tensor_reduce accumulates sum(r*i).
  merge: bn_aggr folds the windows per partition; gpsimd.partition_all_reduce
     folds the 128 partitions (also broadcasts gammas loaded on partition 0).
  consts: inverse-sqrt of the per-channel 2x2 covariance, folded with
     gamma/beta into out = A @ [r, i]^T + b  (A 2x2, b 2-vec per channel).
  pass 2 (apply): stream the shard again, ACT does r*A_rr+b_r / i*A_ii+b_i /
     i*A_ri, DVE does r*A_ir and the two cross adds, DMA out.
"""

import numpy as np

import concourse.bacc as bacc
import concourse.mybir as mybir
import concourse.tile as tile
from concourse.bass_isa import ReduceOp
from concourse.bass_utils import run_bass_kernel_spmd

B, C, H, W = 8, 64, 256, 256
NCORES = 8
CL = C // NCORES              # 8 local channels per core
P = 128                       # SBUF partitions
FPB = H * W * 2 // P          # 1024 floats per partition per (b, c) block
NBLK = B * CL                 # 64 blocks per core
NH = 4                        # batches per DMA tile (half the batch dim)
WIN = 512                     # bn_stats window (HW max)
WPH = NH * FPB // WIN         # 8 windows per half-tile
NWIN = 2 * WPH                # 16 windows per channel
PAIRS = B * H * W             # 524288 (r,i) pairs per channel
EPS = 1e-5
F32 = mybir.dt.float32
ALU = mybir.AluOpType
ACTF = mybir.ActivationFunctionType

_CACHE = {}


def _emit(tc, nc, x, params, y):
    xb = x.rearrange("(b c) p f -> b c p f", c=CL)
    yb = y.rearrange("(b c) p f -> b c p f", c=CL)

    with (
        tc.tile_pool(name="xin", bufs=3) as xin_pool,
        tc.tile_pool(name="xout", bufs=3) as out_pool,
        tc.tile_pool(name="scr", bufs=2) as scr_pool,
        tc.tile_pool(name="st", bufs=1) as st_pool,
    ):
        # persistent stat tiles
        stats = st_pool.tile([P, CL * NWIN * 6], F32)   # bn_stats slots
        ri = st_pool.tile([P, CL * 2], F32)             # ttr accums per (c, half)
        R = st_pool.tile([P, 40], F32)                  # partition-reduce input
        T = st_pool.tile([P, 40], F32)                  # partition-reduce output
        G = st_pool.tile([P, 5 * CL], F32)              # params (pre-broadcast)
        WK = st_pool.tile([P, 40 * CL], F32)            # const-math scratch
        CO = st_pool.tile([P, 6 * CL], F32)             # A_rr A_ri A_ir A_ii b_r b_i

        nc.sync.dma_start(out=G[:, :], in_=params)

        # ---------------- pass 1: statistics ----------------
        for s in range(CL):
            for h in range(2):
                xt = xin_pool.tile([P, NH * FPB], F32, tag="xt")
                nc.sync.dma_start(
                    out=xt[:, :].rearrange("p (n f) -> p n f", f=FPB),
                    in_=xb[h * NH:(h + 1) * NH, s].rearrange("n p f -> p n f"),
                )
                xw = xt[:, :].rearrange("p (w f) -> p w f", f=WIN)
                for w in range(WPH):
                    base = (s * NWIN + h * WPH + w) * 6
                    nc.vector.bn_stats(
                        out=stats[:, base:base + 6], in_=xw[:, w, :]
                    )
                xp = xt[:, :].rearrange("p (n two) -> p n two", two=2)
                prod = scr_pool.tile([P, NH * FPB // 2], F32, tag="prod")
                nc.gpsimd.tensor_mul(out=prod[:, :], in0=xp[:, :, 0],
                                     in1=xp[:, :, 1])
                nc.vector.tensor_reduce(
                    out=ri[:, s * 2 + h:s * 2 + h + 1], in_=prod[:, :],
                    axis=mybir.AxisListType.X, op=ALU.add)

        # ---------------- merge windows + partitions ----------------
        # R cols: [0:8) sum_w mean_r, [8:16) sum_w mean_i,
        # [16:24) sum-of-squares r, [24:32) ss i, [32:40) sum(r*i)
        sv = stats[:, :].rearrange("p (c w k) -> p c w k", c=CL, k=6)
        Me, M2e = sv[:, :, :, 1], sv[:, :, :, 2]
        Mo, M2o = sv[:, :, :, 4], sv[:, :, :, 5]
        SQ = WK[:, 20 * CL:36 * CL].rearrange("p (c w) -> p c w", w=NWIN)
        tA = WK[:, 36 * CL:37 * CL]
        tB = WK[:, 37 * CL:38 * CL]
        AX = mybir.AxisListType.X
        nc.vector.tensor_reduce(out=R[:, 0:CL], in_=Me, axis=AX, op=ALU.add)
        nc.vector.tensor_reduce(out=R[:, CL:2 * CL], in_=Mo, axis=AX, op=ALU.add)
        for par, (Mx, M2x) in enumerate(((Me, M2e), (Mo, M2o))):
            ss = R[:, (2 + par) * CL:(3 + par) * CL]
            nc.vector.tensor_mul(out=SQ, in0=Mx, in1=Mx)
            nc.vector.tensor_reduce(out=tA, in_=SQ, axis=AX, op=ALU.add)
            nc.vector.tensor_reduce(out=tB, in_=M2x, axis=AX, op=ALU.add)
            nc.vector.tensor_scalar_mul(tA, tA, float(WIN // 2))
            nc.vector.tensor_add(out=ss, in0=tA, in1=tB)
        nc.vector.tensor_reduce(
            out=R[:, 4 * CL:5 * CL],
            in_=ri[:, :].rearrange("p (c h) -> p c h", h=2),
            axis=AX,
            op=ALU.add,
        )
        nc.gpsimd.partition_all_reduce(T[:, :], R[:, :], P, ReduceOp.add)

        # ---------------- per-channel constants ----------------
        def wk(i):
            return WK[:, i * CL:(i + 1) * CL]

        g_rr, g_ii, g_ri = G[:, 0:8], G[:, 8:16], G[:, 16:24]
        beta_r, beta_i = G[:, 24:32], G[:, 32:40]
        mu_r, mu_i = wk(0), wk(1)
        E_rr, E_ii, E_ri = wk(2), wk(3), wk(4)
        Vrr, Vii, Vri = wk(5), wk(6), wk(7)
        tau, delta, sq = wk(8), wk(9), wk(10)
        tt, inv, ninv = wk(11), wk(12), wk(13)
        Wrr, Wri, Wii = wk(14), wk(15), wk(16)
        t1, t2 = wk(17), wk(18)

        nc.vector.tensor_scalar_mul(mu_r, T[:, 0:CL], 1.0 / (P * NWIN))
        nc.vector.tensor_scalar_mul(mu_i, T[:, CL:2 * CL], 1.0 / (P * NWIN))
        nc.vector.tensor_scalar_mul(E_rr, T[:, 2 * CL:3 * CL], 1.0 / PAIRS)
        nc.vector.tensor_scalar_mul(E_ii, T[:, 3 * CL:4 * CL], 1.0 / PAIRS)
        nc.vector.tensor_scalar_mul(E_ri, T[:, 4 * CL:5 * CL], 1.0 / PAIRS)
        # V = E[xy] - mu_x mu_y + eps
        nc.vector.tensor_mul(t1, mu_r, mu_r)
        nc.vector.tensor_scalar(out=t1, in0=t1, scalar1=EPS, scalar2=None,
                                op0=ALU.subtract)
        nc.vector.tensor_sub(Vrr, E_rr, t1)
        nc.vector.tensor_mul(t1, mu_i, mu_i)
        nc.vector.tensor_scalar(out=t1, in0=t1, scalar1=EPS, scalar2=None,
                                op0=ALU.subtract)
        nc.vector.tensor_sub(Vii, E_ii, t1)
        nc.vector.tensor_mul(t1, mu_r, mu_i)
        nc.vector.tensor_scalar(out=t1, in0=t1, scalar1=EPS, scalar2=None,
                                op0=ALU.subtract)
        nc.vector.tensor_sub(Vri, E_ri, t1)
        # inverse sqrt of [[Vrr, Vri], [Vri, Vii]]
        nc.vector.tensor_add(tau, Vrr, Vii)
        nc.vector.tensor_mul(t1, Vrr, Vii)
        nc.vector.tensor_mul(t2, Vri, Vri)
        nc.vector.tensor_sub(delta, t1, t2)
        nc.scalar.sqrt(sq, delta)
        nc.vector.tensor_scalar_mul(t1, sq, 2.0)
        nc.vector.tensor_add(t1, t1, tau)
        nc.scalar.sqrt(tt, t1)
        nc.vector.tensor_mul(t1, sq, tt)
        nc.vector.reciprocal(inv, t1)
        nc.vector.tensor_scalar_mul(ninv, inv, -1.0)
        nc.vector.tensor_add(t1, Vii, sq)
        nc.vector.tensor_mul(Wrr, t1, inv)
        nc.vector.tensor_mul(Wri, Vri, ninv)
        nc.vector.tensor_add(t1, Vrr, sq)
        nc.vector.tensor_mul(Wii, t1, inv)
        # fold gamma: A = G @ W   (G = [[g_rr, g_ri], [g_ri, g_ii]])
        A_rr, A_ri = CO[:, 0:CL], CO[:, CL:2 * CL]
        A_ir, A_ii = CO[:, 2 * CL:3 * CL], CO[:, 3 * CL:4 * CL]
        b_r, b_i = CO[:, 4 * CL:5 * CL], CO[:, 5 * CL:6 * CL]
        for dst, ga, wa, gb, wb in (
            (A_rr, g_rr, Wrr, g_ri, Wri),
            (A_ri, g_rr, Wri, g_ri, Wii),
            (A_ir, g_ri, Wrr, g_ii, Wri),
            (A_ii, g_ri, Wri, g_ii, Wii),
        ):
            nc.vector.tensor_mul(t1, ga, wa)
            nc.vector.tensor_mul(t2, gb, wb)
            nc.vector.tensor_add(dst, t1, t2)
        # b = beta - A @ mu
        nc.vector.tensor_mul(t1, A_rr, mu_r)
        nc.vector.tensor_mul(t2, A_ri, mu_i)
        nc.vector.tensor_add(t1, t1, t2)
        nc.vector.tensor_sub(b_r, beta_r, t1)
        nc.vector.tensor_mul(t1, A_ir, mu_r)
        nc.vector.tensor_mul(t2, A_ii, mu_i)
        nc.vector.tensor_add(t1, t1, t2)
        nc.vector.tensor_sub(b_i, beta_i, t1)

        # ---------------- pass 2: apply ----------------
        def col(k, s):
            return CO[:, k * CL + s:k * CL + s + 1]

        for s in range(CL):
            for h in range(2):
                xt = xin_pool.tile([P, NH * FPB], F32, tag="xt")
                nc.sync.dma_start(
                    out=xt[:, :].rearrange("p (n f) -> p n f", f=FPB),
                    in_=xb[h * NH:(h + 1) * NH, s].rearrange("n p f -> p n f"),
                )
                xp = xt[:, :].rearrange("p (n two) -> p n two", two=2)
                xr, xi = xp[:, :, 0], xp[:, :, 1]
                ot = out_pool.tile([P, NH * FPB], F32, tag="ot")
                op = ot[:, :].rearrange("p (n two) -> p n two", two=2)
                o_r, o_i = op[:, :, 0], op[:, :, 1]
                u1 = scr_pool.tile([P, NH * FPB // 2], F32, tag="u1")
                u2 = scr_pool.tile([P, NH * FPB // 2], F32, tag="u2")
                # real: A_rr*r + b_r  (ACT)  +  A_ri*i (ACT)      -> add (DVE)
                # imag: A_ii*i + b_i  (ACT)  +  A_ir*r (DVE TSP)  -> add (DVE)
                nc.scalar.activation(out=o_r, in_=xr, func=ACTF.Identity,
                                     scale=col(0, s), bias=col(4, s))
                nc.scalar.activation(out=u1[:, :], in_=xi, func=ACTF.Copy,
                                     scale=col(1, s))
                nc.scalar.activation(out=o_i, in_=xi, func=ACTF.Identity,
                                     scale=col(3, s), bias=col(5, s))
                nc.vector.tensor_scalar_mul(u2[:, :], xr, col(2, s))
                nc.vector.tensor_add(out=o_r, in0=o_r, in1=u1[:, :])
                nc.vector.tensor_add(out=o_i, in0=o_i, in1=u2[:, :])
                nc.sync.dma_start(
                    out=yb[h * NH:(h + 1) * NH, s].rearrange("n p f -> p n f"),
                    in_=ot[:, :].rearrange("p (n f) -> p n f", f=FPB),
                )


def build():
    if "nc" in _CACHE:
        return _CACHE["nc"]
    nc = bacc.Bacc("TRN2", target_bir_lowering=False, debug=False,
                   num_devices=NCORES)
    x = nc.dram_tensor("x", [NBLK, P, FPB], F32, kind="ExternalInput").ap()
    params = nc.dram_tensor("params", [P, 5 * CL], F32,
                            kind="ExternalInput").ap()
    y = nc.dram_tensor("y", [NBLK, P, FPB], F32, kind="ExternalOutput").ap()
    with tile.TileContext(nc) as tc:
        _emit(tc, nc, x, params, y)
    nc.compile()
    _CACHE["nc"] = nc
    return nc


def make_in_maps(c_input, gamma_rr, gamma_ii, gamma_ri, beta_real, beta_imag):
    c_input = np.ascontiguousarray(np.asarray(c_input, dtype=np.float32))
    in_maps = []
    for m in range(NCORES):
        sl = slice(m * CL, (m + 1) * CL)
        xs = np.ascontiguousarray(
            c_input[:, sl].reshape(NBLK, P, FPB))
        pvec = np.concatenate([
            np.asarray(gamma_rr, np.float32)[sl],
            np.asarray(gamma_ii, np.float32)[sl],
            np.asarray(gamma_ri, np.float32)[sl],
            np.asarray(beta_real, np.float32)[sl],
            np.asarray(beta_imag, np.float32)[sl],
        ])
        ps = np.ascontiguousarray(np.tile(pvec[None, :], (P, 1)))
        in_maps.append({"x": xs, "params": ps})
    return in_maps


def gather(results):
    out = np.empty((B, C, H, W, 2), dtype=np.float32)
    for m in range(NCORES):
        out[:, m * CL:(m + 1) * CL] = results[m]["y"].reshape(B, CL, H, W, 2)
    return out


def run(in_maps, trace=False, **kw):
    nc = build()
    return run_bass_kernel_spmd(nc, in_maps, list(range(NCORES)),
                                trace=trace, **kw)


def kernel(c_input, gamma_rr, gamma_ii, gamma_ri, beta_real, beta_imag):
    in_maps = make_in_maps(c_input, gamma_rr, gamma_ii, gamma_ri,
                           beta_real, beta_imag)
    res = run(in_maps, trace=False)
    return gather(res.results)
